# revision 2
# baseline (speedup 1.0000x reference)
"""MHA Trainium2 kernel v3: single interleaved schedule, (batch x head-group)
sharded across 8 NeuronCores.

Problem: B=2, S=2048, D=2560, H=32 heads, HD=80, partial rotary RD=32,
causal attention, fp32 reference; kernel computes in bf16 with f32 PSUM.

Core c handles batch c//4, heads (c%4)*8 .. +8.

Schedule (the point of v3 is PE density — no phase where PE idles >3us,
so HAM stays at K=8/8):
  V:   v projection st 0..15 -> vA resident (ones channel at col 96 per head
       yields softmax denominator via PV).
  QK:  per st: q,k projection (x stationary), bias, rope, 16 PE transposes
       -> qT/kT [80, s].  After st 4j+3, attention for q-block j becomes
       ready for ALL heads and is drip-fed as filler into the remaining
       projection instruction stream.
  C unit = "exp pair": two consecutive k-tiles' scoresT packed into one
       2-bank PSUM tile (partial-N on diagonal tiles: only cols >= kt*128
       are computed), ONE exp over the packed span, mask multiply only on
       the [128,128] diagonal chunk, two PV accumulates.
  Tail: q0=3 attention with out-proj units (ow loaded into wqk's freed
       SBUF) interleaved as PE filler; host sums partials + bias.
"""
import sys
import os

sys.path.insert(0, "/opt/trn_rl_repo")

import numpy as np
from contextlib import ExitStack
from collections import deque

import concourse.bacc as bacc
import concourse.tile as tile
import concourse.mybir as mybir
from concourse.bass_utils import run_bass_kernel_spmd
from concourse.masks import make_identity

F32 = mybir.dt.float32
F32R = mybir.dt.float32r
BF16 = mybir.dt.bfloat16

B, S, D = 2, 2048, 2560
H, HD = 32, 80
RD = 32
ROPE_BASE = 10000.0
N_CORES = 8
NBG = 4  # cores per batch


def make_cfg(s=S, d=D, nh=H // NBG, hd=HD, rd=RD, qb=512, dt="bf16"):
    cfg = dict(s=s, d=d, nh=nh, hd=hd, rd=rd, qb=qb, dt=dt)
    cfg["n_st"] = s // 128
    cfg["n_kt"] = d // 128
    cfg["nqb"] = s // qb
    cfg["ndiag"] = qb // 128
    cfg["n_dt"] = d // 128
    cfg["jqk"] = 2 * nh * hd          # 1280
    cfg["vw"] = ((hd + 31) // 32) * 32 + 1  # 97: ones channel at 32-aligned col
    cfg["jv"] = nh * cfg["vw"]              # 776 (vA layout, incl. pads)
    cfg["jvc"] = nh * (hd + 1)              # 648 (compact weight cols)
    cfg["n_ct"] = (nh * hd) // 128    # 5 packed ctx tiles
    return cfg


def _dt(cfg):
    return {"bf16": BF16, "f32": F32, "f32r": F32R}[cfg["dt"]]


def build_program(cfg):
    s, d, nh, hd, rd = cfg["s"], cfg["d"], cfg["nh"], cfg["hd"], cfg["rd"]
    qb, n_st, n_kt = cfg["qb"], cfg["n_st"], cfg["n_kt"]
    nqb, ndiag, n_dt = cfg["nqb"], cfg["ndiag"], cfg["n_dt"]
    jqk, vw, jv, n_ct = cfg["jqk"], cfg["vw"], cfg["jv"], cfg["n_ct"]
    jvc = cfg["jvc"]
    DT = _dt(cfg)
    rh = rd // 2
    hw1 = hd + 1

    nc = bacc.Bacc(None, debug=False)

    xs_d = nc.declare_dram_parameter("xs", [n_st, 128, n_kt * 128], DT,
                                     isOutput=False)
    wqk_d = nc.declare_dram_parameter("wqk", [n_kt, 128, jqk], DT,
                                      isOutput=False)
    wv_d = nc.declare_dram_parameter("wv", [n_kt, 128, jvc], DT,
                                     isOutput=False)
    outw_d = nc.declare_dram_parameter("outw", [n_ct, 128, d], DT,
                                       isOutput=False)
    cos_d = nc.declare_dram_parameter("cosR", [128, n_st * nh * rh], DT,
                                      isOutput=False)
    sin_d = nc.declare_dram_parameter("sinR", [128, n_st * nh * rh], DT,
                                      isOutput=False)
    mask_d = nc.declare_dram_parameter("masks", [128, 128], DT,
                                       isOutput=False)
    bqk_d = nc.declare_dram_parameter("bqk", [128, jqk], DT, isOutput=False)
    bv_d = nc.declare_dram_parameter("bv", [128, jvc], DT, isOutput=False)
    y_d = nc.declare_dram_parameter("y", [d, s], DT, isOutput=True)

    qk_groups = [(0, 512), (512, 1024), (1024, 1280)]
    v_groups = [(0, 6 * hw1), (6 * hw1, nh * hw1)]  # head-aligned

    with tile.TileContext(nc) as tc, ExitStack() as top:
        top.enter_context(
            nc.allow_low_precision(reason="intentional bf16 storage"))
        glob = top.enter_context(tc.tile_pool(name="glob", bufs=1))
        identf = glob.tile([128, 128], F32)
        make_identity(nc, identf)
        if DT is F32:
            ident = identf
        else:
            ident = glob.tile([128, 128], DT)
            nc.vector.tensor_copy(ident, identf)
        ones1f = glob.tile([1, hd], F32)
        nc.vector.memset(ones1f, 1.0)
        ones1 = glob.tile([1, hd], F32R)
        nc.vector.tensor_copy(ones1, ones1f)
        cosR = glob.tile([128, n_st * nh * rh], DT)
        sinR = glob.tile([128, n_st * nh * rh], DT)
        mask = glob.tile([128, 128], DT)
        bqk = glob.tile([128, jqk], DT)
        bv = glob.tile([128, jvc], DT)

        qt_pool = top.enter_context(tc.tile_pool(name="qt", bufs=1))
        qT = [qt_pool.tile([hd, s], DT, tag=f"q{h}", name=f"qT{h}")
              for h in range(nh)]
        kT = [qt_pool.tile([hd, s], DT, tag=f"k{h}", name=f"kT{h}")
              for h in range(nh)]
        vp = top.enter_context(tc.tile_pool(name="vp", bufs=1))
        vA = [vp.tile([128, jv], DT, tag=f"v{st}", name=f"vA{st}")
              for st in range(n_st)]
        for st in range(n_st):
            nc.vector.memset(vA[st], 0.0)
        ctx_pool = top.enter_context(tc.tile_pool(name="ctx", bufs=1))
        ctxP = [ctx_pool.tile([128, s], DT, tag=f"cp{t}", name=f"ctxP{t}")
                for t in range(n_ct)]
        xsp = top.enter_context(tc.tile_pool(name="xsp", bufs=2))

        # psM: shared 2-bank PSUM ring — transposes (QK), bcast + out-proj
        # psum (V/tail)
        psM = top.enter_context(
            tc.tile_pool(name="psM", bufs=2, space="PSUM"))
        cstk = ExitStack()

        pctx_live = {}
        uid = [0]

        def emit_c_kt(h, q0, kt, pool_get):
            """One attention unit: scoresT + exp + (diag mask) + PV for one
            k-tile of q-block q0, partial-N on diagonal tiles."""
            u = uid[0]
            uid[0] += 1
            if kt == 0:
                pctx_live[(h, q0)] = psC.tile(
                    [vw, qb], F32, tag="pc", name=f"pctx{h}_{q0}")
            pctx = pctx_live[(h, q0)]
            nkt_q = (q0 + 1) * ndiag
            off = max(0, kt * 128 - q0 * qb)
            n = qb - off
            sgl = pool_get(u)
            nc.tensor.matmul(
                sgl[:, 0:n],
                kT[h][:, kt * 128:(kt + 1) * 128],
                qT[h][:, q0 * qb + off:(q0 + 1) * qb],
                start=True, stop=True)
            pT_ = pp.tile([128, qb], DT, tag="p", name=f"pT{u}")
            nc.scalar.activation(pT_[:, 0:n], sgl[:, 0:n],
                                 mybir.ActivationFunctionType.Exp)
            if kt * 128 >= q0 * qb:
                # diagonal tile: mask the leading [128,128] chunk
                nc.vector.tensor_mul(pT_[:, 0:128], pT_[:, 0:128], mask)
            nc.tensor.matmul(
                pctx[:, off:qb], vA[kt][:, h * vw:(h + 1) * vw],
                pT_[:, 0:n],
                start=(kt == 0), stop=(kt == nkt_q - 1),
                skip_group_check=True)
            if kt == nkt_q - 1:
                finalize(h, q0, pctx)
                del pctx_live[(h, q0)]

        def finalize(h, q0, pctx):
            u = uid[0]
            uid[0] += 1
            den = rp2.tile([1, qb], F32, tag="rd", name=f"rden{u}", bufs=1)
            nc.vector.tensor_copy(den, pctx[vw - 1:vw, :])
            ctx_s = rp2.tile([hd, qb], DT, tag="cs", name=f"cs{u}")
            nc.vector.tensor_copy(ctx_s, pctx[0:hd, :])
            rden = den
            nc.vector.reciprocal_approx_fast(out=rden, in_=den)
            rdenr = rp2.tile([1, qb], F32R, tag="rdr", name=f"rdr{u}", bufs=1)
            nc.vector.tensor_copy(rdenr, rden)
            pbc = psM.tile([hd, qb], F32, tag="m", name=f"pbc{u}")
            nc.tensor.matmul(pbc, ones1, rdenr, start=True, stop=True)
            rb = rp2.tile([hd, qb], DT, tag="rb", name=f"rb{u}")
            nc.vector.tensor_copy(rb, pbc)
            cts = rp2.tile([hd, qb], DT, tag="ctso", name=f"cts{u}")
            nc.vector.tensor_mul(cts, ctx_s, rb)
            # repack into [128, s] ctx tiles
            g0 = h * hd
            r = g0
            c0, c1 = q0 * qb, (q0 + 1) * qb
            while r < g0 + hd:
                ct = r // 128
                r1 = min((ct + 1) * 128, g0 + hd)
                nc.sync.dma_start(
                    out=ctxP[ct][r - ct * 128:r1 - ct * 128, c0:c1],
                    in_=cts[r - g0:r1 - g0, :])
                r = r1

        pending = deque()

        def drip(k=1):
            for _ in range(k):
                if pending:
                    pending.popleft()()

        def main_pool_get(u):
            return psS.tile([128, qb], F32, tag="s", name=f"sc{u}")

        # ---- phase QK: q,k projection + rope + transpose (PE dense) ----
        with ExitStack() as pstk:
            wp = pstk.enter_context(tc.tile_pool(name="w1", bufs=1))
            wqk = [wp.tile([128, jqk], DT, tag=f"w1_{kt}", name=f"wqk{kt}")
                   for kt in range(n_kt)]
            # DMA order matters at startup: st0 inputs first
            xs_pre = []
            for st in range(2):
                xs = xsp.tile([128, n_kt * 128], DT, tag="xs",
                              name=f"xs_qk_{st}")
                xs_pre.append(xs)
            nc.sync.dma_start(out=xs_pre[0], in_=xs_d[0])
            for kt in range(4):
                nc.sync.dma_start(out=wqk[kt], in_=wqk_d[kt])
            nc.sync.dma_start(out=xs_pre[1], in_=xs_d[1])
            for kt in range(4, n_kt):
                nc.sync.dma_start(out=wqk[kt], in_=wqk_d[kt])
            nc.sync.dma_start(out=cosR, in_=cos_d[:, :])
            nc.sync.dma_start(out=sinR, in_=sin_d[:, :])
            nc.sync.dma_start(out=bqk, in_=bqk_d[:, :])
            stp = pstk.enter_context(tc.tile_pool(name="stg1", bufs=3))
            psA = pstk.enter_context(
                tc.tile_pool(name="psA", bufs=2, space="PSUM"))
            rtp = pstk.enter_context(tc.tile_pool(name="rt", bufs=3))
            for st in range(n_st):
                if st < 2:
                    xs = xs_pre[st]
                else:
                    xs = xsp.tile([128, n_kt * 128], DT, tag="xs",
                                  name=f"xs_qk_{st}")
                    nc.sync.dma_start(out=xs, in_=xs_d[st])
                xs3 = xs.rearrange("p (t c) -> p t c", t=n_kt)
                ps = [psA.tile([128, g1 - g0], F32, tag=f"ps{gi}",
                               name=f"psA{st}_{gi}")
                      for gi, (g0, g1) in enumerate(qk_groups)]
                for kt in range(n_kt):
                    for gi, (g0, g1) in enumerate(qk_groups):
                        nc.tensor.matmul(
                            ps[gi], xs3[:, kt, :], wqk[kt][:, g0:g1],
                            start=(kt == 0), stop=(kt == n_kt - 1))
                stage = stp.tile([128, jqk], DT, tag="stage")
                for gi, (g0, g1) in enumerate(qk_groups):
                    nc.vector.scalar_tensor_tensor(
                        out=stage[:, g0:g1], in0=ps[gi], scalar=1.0,
                        in1=bqk[:, g0:g1], op0=mybir.AluOpType.mult,
                        op1=mybir.AluOpType.add)
                # rope: all nh heads per op via strided 3D views
                cN = cosR[:, st * nh * rh:(st + 1) * nh * rh] \
                    .rearrange("p (h c) -> p h c", h=nh)
                sN = sinR[:, st * nh * rh:(st + 1) * nh * rh] \
                    .rearrange("p (h c) -> p h c", h=nh)
                for qk in range(2):
                    blk = stage[:, qk * nh * hd:(qk + 1) * nh * hd] \
                        .rearrange("p (h c) -> p h c", h=nh)
                    t1 = blk[:, :, 0:rh]
                    t2 = blk[:, :, rh:rd]
                    ta = rtp.tile([128, nh, rh], F32, tag="ta")
                    tb = rtp.tile([128, nh, rh], F32, tag="tb")
                    tg = rtp.tile([128, nh, rh], F32, tag="tg")
                    td = rtp.tile([128, nh, rh], F32, tag="td")
                    nc.vector.tensor_mul(ta, t1, cN)
                    nc.vector.tensor_mul(tb, t2, sN)
                    nc.vector.tensor_mul(tg, t1, sN)
                    nc.vector.tensor_mul(td, t2, cN)
                    nc.vector.tensor_sub(t1, ta, tb)
                    nc.vector.tensor_add(t2, tg, td)
                for i in range(2 * nh):  # 16 transposes
                    qk, h = i // nh, i % nh
                    dstT = qT if qk == 0 else kT
                    pt = psM.tile([hd, 128], DT, tag="m",
                                  name=f"pt{st}_{i}")
                    nc.tensor.transpose(
                        pt, stage[:, qk * nh * hd + h * hd:
                                  qk * nh * hd + (h + 1) * hd], ident)
                    nc.vector.tensor_copy(
                        dstT[h][:, st * 128:(st + 1) * 128], pt)

        # ---- attention pools (exist from V phase through the tail) ----
        stp2 = cstk.enter_context(tc.tile_pool(name="st2", bufs=2))
        pp = cstk.enter_context(tc.tile_pool(name="pT", bufs=3))
        rp2 = cstk.enter_context(tc.tile_pool(name="rr", bufs=2))
        psS = cstk.enter_context(
            tc.tile_pool(name="psS", bufs=2, space="PSUM"))
        psC = cstk.enter_context(
            tc.tile_pool(name="psC", bufs=2, space="PSUM"))
        op = cstk.enter_context(tc.tile_pool(name="ow", bufs=1))
        ow = [op.tile([128, d], DT, tag=f"o{t}", name=f"ow{t}")
              for t in range(n_ct)]

        def emit_d_unit(dt_i, sb):
            u = uid[0]
            uid[0] += 1
            psy = psM.tile([128, qb], F32, tag="m", name=f"psy{dt_i}_{sb}")
            for ct in range(n_ct):
                nc.tensor.matmul(
                    psy, ow[ct][:, dt_i * 128:(dt_i + 1) * 128],
                    ctxP[ct][:, sb * qb:(sb + 1) * qb],
                    start=(ct == 0), stop=(ct == n_ct - 1))
            ystage = stp2.tile([128, qb], DT, tag="y",
                               name=f"yst{dt_i}_{sb}")
            nc.vector.tensor_copy(ystage, psy)
            nc.sync.dma_start(
                out=y_d[dt_i * 128:(dt_i + 1) * 128,
                        sb * qb:(sb + 1) * qb],
                in_=ystage)

        # ---- phase V: v projection with attention drip-fed as filler ----
        with ExitStack() as vstk:
            wp2 = vstk.enter_context(tc.tile_pool(name="w2", bufs=1))
            wv = [wp2.tile([128, jvc], DT, tag=f"w2_{kt}", name=f"wv{kt}")
                  for kt in range(n_kt)]
            nc.sync.dma_start(out=bv, in_=bv_d[:, :])
            nc.sync.dma_start(out=mask, in_=mask_d[:, :])
            for kt in range(n_kt):
                nc.sync.dma_start(out=wv[kt], in_=wv_d[kt])
            for t in range(n_ct):
                nc.sync.dma_start(out=ow[t], in_=outw_d[t])
            psA2 = vstk.enter_context(
                tc.tile_pool(name="psA2", bufs=1, space="PSUM"))
            for st in range(n_st):
                xs = xsp.tile([128, n_kt * 128], DT, tag="xs",
                              name=f"xs_v_{st}")
                nc.sync.dma_start(out=xs, in_=xs_d[st])
                xs3 = xs.rearrange("p (t c) -> p t c", t=n_kt)
                ps = [psA2.tile([128, g1 - g0], F32, tag=f"ps{gi}",
                                name=f"psV{st}_{gi}")
                      for gi, (g0, g1) in enumerate(v_groups)]
                for kt in range(n_kt):
                    for gi, (g0, g1) in enumerate(v_groups):
                        nc.tensor.matmul(
                            ps[gi], xs3[:, kt, :], wv[kt][:, g0:g1],
                            start=(kt == 0), stop=(kt == n_kt - 1))
                    drip(1)
                vA3 = vA[st].rearrange("p (h c) -> p h c", h=nh)
                for gi, (g0, g1) in enumerate(v_groups):
                    h0, h1 = g0 // hw1, g1 // hw1
                    ps3 = ps[gi].rearrange("p (h c) -> p h c", h=h1 - h0)
                    bv3 = bv[:, g0:g1].rearrange("p (h c) -> p h c",
                                                 h=h1 - h0)
                    nc.vector.scalar_tensor_tensor(
                        out=vA3[:, h0:h1, 0:hd], in0=ps3[:, :, 0:hd],
                        scalar=1.0, in1=bv3[:, :, 0:hd],
                        op0=mybir.AluOpType.mult, op1=mybir.AluOpType.add)
                    nc.vector.scalar_tensor_tensor(
                        out=vA3[:, h0:h1, vw - 1:vw],
                        in0=ps3[:, :, hd:hd + 1], scalar=1.0,
                        in1=bv3[:, :, hd:hd + 1],
                        op0=mybir.AluOpType.mult, op1=mybir.AluOpType.add)
                    drip(1)
                if st % ndiag == ndiag - 1:
                    q0 = st // ndiag
                    if q0 < nqb - 1:
                        for h in range(nh):
                            for kt in range((q0 + 1) * ndiag):
                                pending.append(
                                    lambda h=h, q0=q0, kt=kt:
                                    emit_c_kt(h, q0, kt, main_pool_get))
                if st % ndiag == 0 and st >= 2 * ndiag:
                    sb = st // ndiag - 2  # out-proj for completed q-blocks
                    pending.extend(
                        lambda dt_i=dt_i, sb=sb: emit_d_unit(dt_i, sb)
                        for dt_i in range(n_dt))
        # drain any leftover pre-tail units
        while pending:
            drip(1)

        # ---- tail: q0 = nqb-1 attention + out-proj drip ----
        with ExitStack() as tstk:
            tps = tstk.enter_context(
                tc.tile_pool(name="tps", bufs=2, space="PSUM"))
            tcnt = [0]

            def tail_pool_get(u):
                tcnt[0] += 1
                if tcnt[0] % 2 == 0:
                    return psS.tile([128, qb], F32, tag="s", name=f"sc{u}")
                return tps.tile([128, qb], F32, tag="s2", name=f"sc{u}")

            q0 = nqb - 1
            cunits = [
                lambda h=h, q0=q0, kt=kt:
                emit_c_kt(h, q0, kt, tail_pool_get)
                for h in range(nh) for kt in range((q0 + 1) * ndiag)
            ]
            dunits = deque((dt_i, nqb - 2) for dt_i in range(n_dt))
            for i, cu in enumerate(cunits):
                cu()
                if i % 4 == 0 and dunits:
                    emit_d_unit(*dunits.popleft())
            while dunits:
                emit_d_unit(*dunits.popleft())
            for dt_i in range(n_dt):
                emit_d_unit(dt_i, nqb - 1)
        cstk.close()

    nc.finalize()
    return nc


def prep_core_inputs(cfg, x, wqkv_w, wqkv_b, out_w, core):
    s, d, nh, hd, rd = cfg["s"], cfg["d"], cfg["nh"], cfg["hd"], cfg["rd"]
    qb, n_st, n_kt = cfg["qb"], cfg["n_st"], cfg["n_kt"]
    ndiag, jqk, vw, jv = cfg["ndiag"], cfg["jqk"], cfg["vw"], cfg["jv"]
    n_ct = cfg["n_ct"]
    rh = rd // 2
    npdt = mybir.dt.np(_dt(cfg))

    bi = core // NBG
    hg = core % NBG
    heads = range(hg * nh, (hg + 1) * nh)
    rows = np.concatenate([np.arange(h * hd, (h + 1) * hd) for h in heads])
    scale = np.float32(1.0 / np.sqrt(hd))

    wq = wqkv_w[rows, :]
    bq = wqkv_b[rows]
    wk = wqkv_w[d + rows, :] * scale
    bk = wqkv_b[d + rows] * scale
    wv = wqkv_w[2 * d + rows, :]
    bv = wqkv_b[2 * d + rows]

    def wt_tiles(w):
        return np.ascontiguousarray(w.T).reshape(n_kt, 128, w.shape[0])

    wqk_arr = np.concatenate([wt_tiles(wq), wt_tiles(wk)], axis=2)
    bqk_arr = np.broadcast_to(
        np.concatenate([bq, bk])[None, :], (128, jqk))

    # v: compact 81 cols per head (80 weights + ones channel with zero
    # weights and bias 1); the device copies re-stride into the vA layout.
    jvc = cfg["jvc"]
    hw1 = hd + 1
    wva = np.zeros((d, jvc), np.float32)
    bva = np.zeros((jvc,), np.float32)
    for h in range(nh):
        wva[:, h * hw1:h * hw1 + hd] = wv[h * hd:(h + 1) * hd].T
        bva[h * hw1:h * hw1 + hd] = bv[h * hd:(h + 1) * hd]
        bva[h * hw1 + hd] = 1.0
    wv_arr = wva.reshape(n_kt, 128, jvc)
    bv_arr = np.broadcast_to(bva[None, :], (128, jvc))

    outw_arr = np.ascontiguousarray(
        out_w[:, rows].T.reshape(n_ct, 128, d))

    inv_freq = 1.0 / (ROPE_BASE ** (np.arange(0, rd, 2, dtype=np.float32) / rd))
    t = np.arange(s, dtype=np.float32)
    freqs = np.outer(t, inv_freq)  # [s, rh]
    # [128, n_st, nh, rh]: value depends on (token=st*128+p, freq i); repl. nh
    cos_arr = np.cos(freqs).astype(np.float32).reshape(n_st, 128, rh)
    cos_arr = np.broadcast_to(cos_arr[:, :, None, :], (n_st, 128, nh, rh))
    cos_arr = np.ascontiguousarray(
        cos_arr.transpose(1, 0, 2, 3).reshape(128, n_st * nh * rh))
    sin_arr = np.sin(freqs).astype(np.float32).reshape(n_st, 128, rh)
    sin_arr = np.broadcast_to(sin_arr[:, :, None, :], (n_st, 128, nh, rh))
    sin_arr = np.ascontiguousarray(
        sin_arr.transpose(1, 0, 2, 3).reshape(128, n_st * nh * rh))

    km = np.arange(128)[:, None]
    qm = np.arange(128)[None, :]
    mask_arr = (qm >= km).astype(np.float32)

    xa = np.ascontiguousarray(x[bi].T)
    xs_arr = np.ascontiguousarray(
        xa.reshape(n_kt, 128, n_st, 128).transpose(2, 1, 0, 3)
    ).reshape(n_st, 128, n_kt * 128)

    return {
        "xs": xs_arr.astype(npdt),
        "wqk": np.ascontiguousarray(wqk_arr).astype(npdt),
        "wv": np.ascontiguousarray(wv_arr).astype(npdt),
        "outw": outw_arr.astype(npdt),
        "cosR": cos_arr.astype(npdt),
        "sinR": sin_arr.astype(npdt),
        "masks": np.ascontiguousarray(mask_arr).astype(npdt),
        "bqk": np.ascontiguousarray(bqk_arr).astype(npdt),
        "bv": np.ascontiguousarray(bv_arr).astype(npdt),
    }


_CACHE = {}


def run_mha(cfg, x, wqkv_w, wqkv_b, out_w, out_b, trace=False):
    key = tuple(sorted(cfg.items()))
    if key not in _CACHE:
        _CACHE[key] = build_program(cfg)
    nc = _CACHE[key]
    in_maps = [
        prep_core_inputs(cfg, x, wqkv_w, wqkv_b, out_w, c)
        for c in range(N_CORES)
    ]
    res = run_bass_kernel_spmd(nc, in_maps, core_ids=list(range(N_CORES)),
                               trace=trace)
    d, s = cfg["d"], cfg["s"]
    y = np.zeros((B, s, d), np.float32)
    for bi in range(B):
        acc = np.zeros((d, s), np.float32)
        for c in range(bi * NBG, (bi + 1) * NBG):
            acc += res.results[c]["y"].astype(np.float32)
        y[bi] = acc.T + out_b[None, :]
    return y, res


def kernel(x, wqkv_w, wqkv_b, out_w, out_b):
    cfg = make_cfg(dt=os.environ.get("KMHA_DT", "bf16"))
    y, _ = run_mha(cfg, np.asarray(x, np.float32),
                   np.asarray(wqkv_w, np.float32),
                   np.asarray(wqkv_b, np.float32),
                   np.asarray(out_w, np.float32),
                   np.asarray(out_b, np.float32))
    return y


# revision 10
# speedup vs baseline: 1.0491x; 1.0491x over previous
"""MHA Trainium2 kernel v3: single interleaved schedule, (batch x head-group)
sharded across 8 NeuronCores.

Problem: B=2, S=2048, D=2560, H=32 heads, HD=80, partial rotary RD=32,
causal attention, fp32 reference; kernel computes in bf16 with f32 PSUM.

Core c handles batch c//4, heads (c%4)*8 .. +8.

Schedule (the point of v3 is PE density — no phase where PE idles >3us,
so HAM stays at K=8/8):
  V:   v projection st 0..15 -> vA resident (ones channel at col 96 per head
       yields softmax denominator via PV).
  QK:  per st: q,k projection (x stationary), bias, rope, 16 PE transposes
       -> qT/kT [80, s].  After st 4j+3, attention for q-block j becomes
       ready for ALL heads and is drip-fed as filler into the remaining
       projection instruction stream.
  C unit = "exp pair": two consecutive k-tiles' scoresT packed into one
       2-bank PSUM tile (partial-N on diagonal tiles: only cols >= kt*128
       are computed), ONE exp over the packed span, mask multiply only on
       the [128,128] diagonal chunk, two PV accumulates.
  Tail: q0=3 attention with out-proj units (ow loaded into wqk's freed
       SBUF) interleaved as PE filler; host sums partials + bias.
"""
import sys
import os

sys.path.insert(0, "/opt/trn_rl_repo")

import numpy as np
from contextlib import ExitStack
from collections import deque

import concourse.bacc as bacc
import concourse.tile as tile
import concourse.mybir as mybir
from concourse.bass_utils import run_bass_kernel_spmd
from concourse.masks import make_identity

F32 = mybir.dt.float32
F32R = mybir.dt.float32r
BF16 = mybir.dt.bfloat16

B, S, D = 2, 2048, 2560
H, HD = 32, 80
RD = 32
ROPE_BASE = 10000.0
N_CORES = 8
NBG = 4  # cores per batch


def make_cfg(s=S, d=D, nh=H // NBG, hd=HD, rd=RD, qb=512, dt="bf16"):
    cfg = dict(s=s, d=d, nh=nh, hd=hd, rd=rd, qb=qb, dt=dt)
    cfg["n_st"] = s // 128
    cfg["n_kt"] = d // 128
    cfg["nqb"] = s // qb
    cfg["ndiag"] = qb // 128
    cfg["n_dt"] = d // 128
    cfg["jqk"] = 2 * nh * hd          # 1280
    cfg["vw"] = ((hd + 31) // 32) * 32 + 1  # 97: ones channel at 32-aligned col
    cfg["jv"] = nh * cfg["vw"]              # 776 (vA layout, incl. pads)
    cfg["jvc"] = nh * (hd + 1)              # 648 (compact weight cols)
    cfg["n_ct"] = (nh * hd) // 128    # 5 packed ctx tiles
    return cfg


def _dt(cfg):
    return {"bf16": BF16, "f32": F32, "f32r": F32R}[cfg["dt"]]


def build_program(cfg):
    s, d, nh, hd, rd = cfg["s"], cfg["d"], cfg["nh"], cfg["hd"], cfg["rd"]
    qb, n_st, n_kt = cfg["qb"], cfg["n_st"], cfg["n_kt"]
    nqb, ndiag, n_dt = cfg["nqb"], cfg["ndiag"], cfg["n_dt"]
    jqk, vw, jv, n_ct = cfg["jqk"], cfg["vw"], cfg["jv"], cfg["n_ct"]
    jvc = cfg["jvc"]
    DT = _dt(cfg)
    rh = rd // 2
    hw1 = hd + 1

    nc = bacc.Bacc(None, debug=False)

    xs_d = nc.declare_dram_parameter("xs", [n_st, 128, n_kt * 128], DT,
                                     isOutput=False)
    wqk_d = nc.declare_dram_parameter("wqk", [n_kt, 128, jqk], DT,
                                      isOutput=False)
    wv_d = nc.declare_dram_parameter("wv", [n_kt, 128, jvc], DT,
                                     isOutput=False)
    outw_d = nc.declare_dram_parameter("outw", [n_ct, 128, d], DT,
                                       isOutput=False)
    cos_d = nc.declare_dram_parameter("cosR", [128, n_st * nh * rh], DT,
                                      isOutput=False)
    sin_d = nc.declare_dram_parameter("sinR", [128, n_st * nh * rh], DT,
                                      isOutput=False)
    mask_d = nc.declare_dram_parameter("masks", [128, 128], DT,
                                       isOutput=False)
    bqk_d = nc.declare_dram_parameter("bqk", [128, jqk], DT, isOutput=False)
    bv_d = nc.declare_dram_parameter("bv", [128, jvc], DT, isOutput=False)
    y_d = nc.declare_dram_parameter("y", [d, s], DT, isOutput=True)

    qk_groups = [(0, 512), (512, 1024), (1024, 1280)]
    v_groups = [(0, 6 * hw1), (6 * hw1, nh * hw1)]  # head-aligned

    with tile.TileContext(nc) as tc, ExitStack() as top:
        top.enter_context(
            nc.allow_low_precision(reason="intentional bf16 storage"))
        glob = top.enter_context(tc.tile_pool(name="glob", bufs=1))
        identf = glob.tile([128, 128], F32)
        make_identity(nc, identf)
        if DT is F32:
            ident = identf
        else:
            ident = glob.tile([128, 128], DT)
            nc.vector.tensor_copy(ident, identf)
        ones1f = glob.tile([1, hd], F32)
        nc.vector.memset(ones1f, 1.0)
        ones1 = glob.tile([1, hd], F32R)
        nc.vector.tensor_copy(ones1, ones1f)
        cosR = glob.tile([128, n_st * nh * rh], DT)
        sinR = glob.tile([128, n_st * nh * rh], DT)
        mask = glob.tile([128, 128], DT)
        bqk = glob.tile([128, jqk], DT)
        bv = glob.tile([128, jvc], DT)

        warmT = glob.tile([1, 16], F32)
        nc.vector.memset(warmT, 0.0)
        warmO = glob.tile([1, 16], DT)
        nc.scalar.activation(warmO, warmT,
                             mybir.ActivationFunctionType.Exp)

        qt_pool = top.enter_context(tc.tile_pool(name="qt", bufs=1))
        qT = [qt_pool.tile([hd, s], DT, tag=f"q{h}", name=f"qT{h}")
              for h in range(nh)]
        kT = [qt_pool.tile([hd, s], DT, tag=f"k{h}", name=f"kT{h}")
              for h in range(nh)]
        vp = top.enter_context(tc.tile_pool(name="vp", bufs=1))
        vA = [vp.tile([128, jv], DT, tag=f"v{st}", name=f"vA{st}")
              for st in range(n_st)]
        for st in range(n_st):
            nc.vector.memset(vA[st], 0.0)
        ctx_pool = top.enter_context(tc.tile_pool(name="ctx", bufs=1))
        ctxP = [ctx_pool.tile([128, s], DT, tag=f"cp{t}", name=f"ctxP{t}")
                for t in range(n_ct)]
        xsp = top.enter_context(tc.tile_pool(name="xsp", bufs=2))

        # psM: shared 2-bank PSUM ring — transposes (QK), bcast + out-proj
        # psum (V/tail)
        psM = top.enter_context(
            tc.tile_pool(name="psM", bufs=2, space="PSUM"))
        cstk = ExitStack()

        pctx_live = {}
        uid = [0]

        def emit_c_kt(h, q0, kt, pool_get):
            """One attention unit: scoresT + exp + (diag mask) + PV for one
            k-tile of q-block q0, partial-N on diagonal tiles."""
            u = uid[0]
            uid[0] += 1
            if kt == 0:
                pctx_live[(h, q0)] = psC.tile(
                    [vw, qb], F32, tag="pc", name=f"pctx{h}_{q0}")
            pctx = pctx_live[(h, q0)]
            nkt_q = (q0 + 1) * ndiag
            off = max(0, kt * 128 - q0 * qb)
            n = qb - off
            sgl = pool_get(u)
            nc.tensor.matmul(
                sgl[:, 0:n],
                kT[h][:, kt * 128:(kt + 1) * 128],
                qT[h][:, q0 * qb + off:(q0 + 1) * qb],
                start=True, stop=True)
            pT_ = pp.tile([128, qb], DT, tag="p", name=f"pT{u}")
            nc.scalar.activation(pT_[:, 0:n], sgl[:, 0:n],
                                 mybir.ActivationFunctionType.Exp)
            if kt * 128 >= q0 * qb:
                # diagonal tile: mask the leading [128,128] chunk
                nc.vector.tensor_mul(pT_[:, 0:128], pT_[:, 0:128], mask)
            nc.tensor.matmul(
                pctx[:, off:qb], vA[kt][:, h * vw:(h + 1) * vw],
                pT_[:, 0:n],
                start=(kt == 0), stop=(kt == nkt_q - 1),
                skip_group_check=True)
            if kt == nkt_q - 1:
                finalize(h, q0, pctx)
                del pctx_live[(h, q0)]

        fin_q = deque()

        def finalize(h, q0, pctx):
            # phase A: drain PSUM, reciprocal, kick the GpSimd broadcast.
            # The normalize (DVE) is deferred via fin_q so it never parks at
            # the DVE queue head waiting out the ~5us broadcast latency.
            u = uid[0]
            uid[0] += 1
            den = rp2.tile([1, qb], F32, tag="rd", name=f"rden{u}", bufs=1)
            nc.vector.tensor_copy(den, pctx[vw - 1:vw, :])
            ctx_s = rp2.tile([hd, qb], DT, tag="cs", name=f"cs{u}", bufs=3)
            nc.vector.tensor_copy(ctx_s, pctx[0:hd, :])
            rden = den
            nc.vector.reciprocal_approx_fast(out=rden, in_=den)
            rdenb = rp2.tile([1, qb], DT, tag="rdb", name=f"rdb{u}", bufs=3)
            nc.vector.tensor_copy(rdenb, rden)
            rb = rp2.tile([hd, qb], DT, tag="rb", name=f"rb{u}", bufs=3)
            nc.gpsimd.partition_broadcast(rb, rdenb, channels=hd)
            fin_q.append(lambda: finalize_b(h, q0, ctx_s, rb))

        def finalize_b(h, q0, ctx_s, rb):
            u = uid[0]
            uid[0] += 1
            cts = rp2.tile([hd, qb], DT, tag="ctso", name=f"cts{u}")
            nc.vector.tensor_mul(cts, ctx_s, rb)
            g0 = h * hd
            r = g0
            c0, c1 = q0 * qb, (q0 + 1) * qb
            while r < g0 + hd:
                ct = r // 128
                r1 = min((ct + 1) * 128, g0 + hd)
                nc.sync.dma_start(
                    out=ctxP[ct][r - ct * 128:r1 - ct * 128, c0:c1],
                    in_=cts[r - g0:r1 - g0, :])
                r = r1

        pending = deque()

        def drip(k=1):
            for _ in range(k):
                if len(fin_q) >= 2:
                    fin_q.popleft()()
                elif pending:
                    pending.popleft()()
                elif fin_q:
                    fin_q.popleft()()

        def main_pool_get(u):
            return psS.tile([128, qb], F32, tag="s", name=f"sc{u}")

        # small prefetch pool so V-phase weights start landing during QK
        w2pre_pool = top.enter_context(tc.tile_pool(name="w2pre", bufs=1))
        wv_pre = [w2pre_pool.tile([128, jvc], DT, tag=f"w2p_{kt}",
                                  name=f"wvp{kt}")
                  for kt in range(8)]

        # ---- phase QK: q,k projection + rope + transpose (PE dense) ----
        with ExitStack() as pstk:
            wp = pstk.enter_context(tc.tile_pool(name="w1", bufs=1))
            wqk = [wp.tile([128, jqk], DT, tag=f"w1_{kt}", name=f"wqk{kt}")
                   for kt in range(n_kt)]
            # DMA order matters at startup: st0 inputs first
            xs_pre = []
            for st in range(2):
                xs = xsp.tile([128, n_kt * 128], DT, tag="xs",
                              name=f"xs_qk_{st}")
                xs_pre.append(xs)
            nc.sync.dma_start(out=xs_pre[0][:, 0:512], in_=xs_d[0][:, 0:512])
            nc.sync.dma_start(out=wqk[0][:, 0:512], in_=wqk_d[0][:, 0:512])
            nc.sync.dma_start(out=wqk[0][:, 512:jqk], in_=wqk_d[0][:, 512:jqk])
            nc.sync.dma_start(out=xs_pre[0][:, 512:n_kt * 128],
                              in_=xs_d[0][:, 512:n_kt * 128])
            for kt in range(1, 4):
                nc.sync.dma_start(out=wqk[kt], in_=wqk_d[kt])
            nc.sync.dma_start(out=xs_pre[1], in_=xs_d[1])
            for kt in range(4, n_kt):
                nc.sync.dma_start(out=wqk[kt], in_=wqk_d[kt])
            nc.sync.dma_start(out=cosR, in_=cos_d[:, :])
            nc.sync.dma_start(out=sinR, in_=sin_d[:, :])
            nc.sync.dma_start(out=bqk, in_=bqk_d[:, :])
            stp = pstk.enter_context(tc.tile_pool(name="stg1", bufs=3))
            psA = pstk.enter_context(
                tc.tile_pool(name="psA", bufs=2, space="PSUM"))
            rtp = pstk.enter_context(tc.tile_pool(name="rt", bufs=3))
            for st in range(n_st):
                if st < 2:
                    xs = xs_pre[st]
                else:
                    xs = xsp.tile([128, n_kt * 128], DT, tag="xs",
                                  name=f"xs_qk_{st}")
                    nc.sync.dma_start(out=xs, in_=xs_d[st])
                if 8 <= st < 8 + len(wv_pre):
                    nc.sync.dma_start(out=wv_pre[st - 8],
                                      in_=wv_d[st - 8])
                xs3 = xs.rearrange("p (t c) -> p t c", t=n_kt)
                ps = [psA.tile([128, g1 - g0], F32, tag=f"ps{gi}",
                               name=f"psA{st}_{gi}")
                      for gi, (g0, g1) in enumerate(qk_groups)]
                for kt in range(n_kt):
                    for gi, (g0, g1) in enumerate(qk_groups):
                        nc.tensor.matmul(
                            ps[gi], xs3[:, kt, :], wqk[kt][:, g0:g1],
                            start=(kt == 0), stop=(kt == n_kt - 1))
                stage = stp.tile([128, jqk], DT, tag="stage")
                for gi, (g0, g1) in enumerate(qk_groups):
                    nc.vector.scalar_tensor_tensor(
                        out=stage[:, g0:g1], in0=ps[gi], scalar=1.0,
                        in1=bqk[:, g0:g1], op0=mybir.AluOpType.mult,
                        op1=mybir.AluOpType.add)
                # rope: all nh heads per op via strided 3D views
                cN = cosR[:, st * nh * rh:(st + 1) * nh * rh] \
                    .rearrange("p (h c) -> p h c", h=nh)
                sN = sinR[:, st * nh * rh:(st + 1) * nh * rh] \
                    .rearrange("p (h c) -> p h c", h=nh)
                for qk in range(2):
                    blk = stage[:, qk * nh * hd:(qk + 1) * nh * hd] \
                        .rearrange("p (h c) -> p h c", h=nh)
                    t1 = blk[:, :, 0:rh]
                    t2 = blk[:, :, rh:rd]
                    ta = rtp.tile([128, nh, rh], F32, tag="ta")
                    tb = rtp.tile([128, nh, rh], F32, tag="tb")
                    tg = rtp.tile([128, nh, rh], F32, tag="tg")
                    td = rtp.tile([128, nh, rh], F32, tag="td")
                    nc.vector.tensor_mul(ta, t1, cN)
                    nc.vector.tensor_mul(tb, t2, sN)
                    nc.vector.tensor_mul(tg, t1, sN)
                    nc.vector.tensor_mul(td, t2, cN)
                    nc.vector.tensor_sub(t1, ta, tb)
                    nc.vector.tensor_add(t2, tg, td)
                for i in range(2 * nh):  # 16 transposes
                    qk, h = i // nh, i % nh
                    dstT = qT if qk == 0 else kT
                    pt = psM.tile([hd, 128], DT, tag="m",
                                  name=f"pt{st}_{i}")
                    nc.tensor.transpose(
                        pt, stage[:, qk * nh * hd + h * hd:
                                  qk * nh * hd + (h + 1) * hd], ident)
                    nc.scalar.copy(
                        dstT[h][:, st * 128:(st + 1) * 128], pt)

        # ---- attention pools (exist from V phase through the tail) ----
        stp2 = cstk.enter_context(tc.tile_pool(name="st2", bufs=2))
        pp = cstk.enter_context(tc.tile_pool(name="pT", bufs=3))
        rp2 = cstk.enter_context(tc.tile_pool(name="rr", bufs=2))
        psS = cstk.enter_context(
            tc.tile_pool(name="psS", bufs=2, space="PSUM"))
        psC = cstk.enter_context(
            tc.tile_pool(name="psC", bufs=2, space="PSUM"))
        op = cstk.enter_context(tc.tile_pool(name="ow", bufs=1))
        ow = [op.tile([128, d], DT, tag=f"o{t}", name=f"ow{t}")
              for t in range(n_ct)]

        def emit_d_unit(dt_i, sb, psy_get=None):
            u = uid[0]
            uid[0] += 1
            if psy_get is None:
                psy = psM.tile([128, qb], F32, tag="m",
                               name=f"psy{dt_i}_{sb}")
            else:
                psy = psy_get(f"psy{dt_i}_{sb}")
            for ct in range(n_ct):
                nc.tensor.matmul(
                    psy, ow[ct][:, dt_i * 128:(dt_i + 1) * 128],
                    ctxP[ct][:, sb * qb:(sb + 1) * qb],
                    start=(ct == 0), stop=(ct == n_ct - 1))
            ystage = stp2.tile([128, qb], DT, tag="y",
                               name=f"yst{dt_i}_{sb}")
            nc.vector.tensor_copy(ystage, psy)
            nc.sync.dma_start(
                out=y_d[dt_i * 128:(dt_i + 1) * 128,
                        sb * qb:(sb + 1) * qb],
                in_=ystage)

        # ---- phase V: v projection with attention drip-fed as filler ----
        with ExitStack() as vstk:
            wp2 = vstk.enter_context(tc.tile_pool(name="w2", bufs=1))
            wv = [wp2.tile([128, jvc], DT, tag=f"w2_{kt}", name=f"wv{kt}")
                  if kt >= len(wv_pre) else wv_pre[kt]
                  for kt in range(n_kt)]
            # x tiles for st 0..1 must not queue behind the weight DMAs
            xs_vpre = []
            for st in range(2):
                xsv = xsp.tile([128, n_kt * 128], DT, tag="xs",
                               name=f"xs_v_{st}")
                xs_vpre.append(xsv)
            nc.sync.dma_start(out=xs_vpre[0], in_=xs_d[0])
            nc.sync.dma_start(out=bv, in_=bv_d[:, :])
            nc.sync.dma_start(out=xs_vpre[1], in_=xs_d[1])
            for kt in range(len(wv_pre), n_kt):
                nc.sync.dma_start(out=wv[kt], in_=wv_d[kt])
            nc.sync.dma_start(out=mask, in_=mask_d[:, :])
            psA2 = vstk.enter_context(
                tc.tile_pool(name="psA2", bufs=1, space="PSUM"))
            for st in range(n_st):
                if st < 2:
                    xs = xs_vpre[st]
                else:
                    xs = xsp.tile([128, n_kt * 128], DT, tag="xs",
                                  name=f"xs_v_{st}")
                    nc.sync.dma_start(out=xs, in_=xs_d[st])
                if 3 <= st < 3 + n_ct:
                    # ow needed from the st8 out-proj drip onward; one tile
                    # per boundary so xs prefetches never queue behind it
                    nc.sync.dma_start(out=ow[st - 3], in_=outw_d[st - 3])
                xs3 = xs.rearrange("p (t c) -> p t c", t=n_kt)
                ps = [psA2.tile([128, g1 - g0], F32, tag=f"ps{gi}",
                                name=f"psV{st}_{gi}")
                      for gi, (g0, g1) in enumerate(v_groups)]
                for kt in range(n_kt):
                    for gi, (g0, g1) in enumerate(v_groups):
                        nc.tensor.matmul(
                            ps[gi], xs3[:, kt, :], wv[kt][:, g0:g1],
                            start=(kt == 0), stop=(kt == n_kt - 1))
                    drip(1 if len(pending) < 32 else 2)
                vA3 = vA[st].rearrange("p (h c) -> p h c", h=nh)
                for gi, (g0, g1) in enumerate(v_groups):
                    h0, h1 = g0 // hw1, g1 // hw1
                    ps3 = ps[gi].rearrange("p (h c) -> p h c", h=h1 - h0)
                    bv3 = bv[:, g0:g1].rearrange("p (h c) -> p h c",
                                                 h=h1 - h0)
                    nc.vector.scalar_tensor_tensor(
                        out=vA3[:, h0:h1, 0:hd], in0=ps3[:, :, 0:hd],
                        scalar=1.0, in1=bv3[:, :, 0:hd],
                        op0=mybir.AluOpType.mult, op1=mybir.AluOpType.add)
                    nc.vector.scalar_tensor_tensor(
                        out=vA3[:, h0:h1, vw - 1:vw],
                        in0=ps3[:, :, hd:hd + 1], scalar=1.0,
                        in1=bv3[:, :, hd:hd + 1],
                        op0=mybir.AluOpType.mult, op1=mybir.AluOpType.add)
                    drip(1)
                if st % ndiag == ndiag - 1:
                    q0 = st // ndiag
                    if q0 < nqb - 1:
                        for h in range(nh):
                            for kt in range((q0 + 1) * ndiag):
                                pending.append(
                                    lambda h=h, q0=q0, kt=kt:
                                    emit_c_kt(h, q0, kt, main_pool_get))
                if st % ndiag == 0 and st >= 2 * ndiag:
                    sb = st // ndiag - 2  # out-proj for completed q-blocks
                    pending.extend(
                        lambda dt_i=dt_i, sb=sb: emit_d_unit(dt_i, sb)
                        for dt_i in range(n_dt))
        # drain any leftover pre-tail units
        while pending:
            drip(1)

        # ---- tail: q0 = nqb-1 attention + out-proj drip ----
        with ExitStack() as tstk:
            tps = tstk.enter_context(
                tc.tile_pool(name="tps", bufs=2, space="PSUM"))
            tcnt = [0]

            def tail_pool_get(u):
                tcnt[0] += 1
                if tcnt[0] % 2 == 0:
                    return psS.tile([128, qb], F32, tag="s", name=f"sc{u}")
                return tps.tile([128, qb], F32, tag="s2", name=f"sc{u}")

            q0 = nqb - 1
            cunits = [
                lambda h=h, q0=q0, kt=kt:
                emit_c_kt(h, q0, kt, tail_pool_get)
                for h in range(nh) for kt in range((q0 + 1) * ndiag)
            ]
            dunits = deque((dt_i, nqb - 2) for dt_i in range(n_dt))
            for i, cu in enumerate(cunits):
                cu()
                if i % 4 == 0 and dunits:
                    emit_d_unit(*dunits.popleft())
                drip(1)
            while dunits:
                emit_d_unit(*dunits.popleft())
            while fin_q or pending:
                drip(1)
            rings = [
                lambda nm: psM.tile([128, qb], F32, tag="m", name=nm),
                lambda nm: tps.tile([128, qb], F32, tag="s2", name=nm),
                lambda nm: psS.tile([128, qb], F32, tag="s", name=nm),
            ]
            for dt_i in range(n_dt):
                emit_d_unit(dt_i, nqb - 1, rings[dt_i % 3])
        cstk.close()

    nc.finalize()
    return nc


def prep_core_inputs(cfg, x, wqkv_w, wqkv_b, out_w, core):
    s, d, nh, hd, rd = cfg["s"], cfg["d"], cfg["nh"], cfg["hd"], cfg["rd"]
    qb, n_st, n_kt = cfg["qb"], cfg["n_st"], cfg["n_kt"]
    ndiag, jqk, vw, jv = cfg["ndiag"], cfg["jqk"], cfg["vw"], cfg["jv"]
    n_ct = cfg["n_ct"]
    rh = rd // 2
    npdt = mybir.dt.np(_dt(cfg))

    bi = core // NBG
    hg = core % NBG
    heads = range(hg * nh, (hg + 1) * nh)
    rows = np.concatenate([np.arange(h * hd, (h + 1) * hd) for h in heads])
    scale = np.float32(1.0 / np.sqrt(hd))

    wq = wqkv_w[rows, :]
    bq = wqkv_b[rows]
    wk = wqkv_w[d + rows, :] * scale
    bk = wqkv_b[d + rows] * scale
    wv = wqkv_w[2 * d + rows, :]
    bv = wqkv_b[2 * d + rows]

    def wt_tiles(w):
        return np.ascontiguousarray(w.T).reshape(n_kt, 128, w.shape[0])

    wqk_arr = np.concatenate([wt_tiles(wq), wt_tiles(wk)], axis=2)
    bqk_arr = np.broadcast_to(
        np.concatenate([bq, bk])[None, :], (128, jqk))

    # v: compact 81 cols per head (80 weights + ones channel with zero
    # weights and bias 1); the device copies re-stride into the vA layout.
    jvc = cfg["jvc"]
    hw1 = hd + 1
    wva = np.zeros((d, jvc), np.float32)
    bva = np.zeros((jvc,), np.float32)
    for h in range(nh):
        wva[:, h * hw1:h * hw1 + hd] = wv[h * hd:(h + 1) * hd].T
        bva[h * hw1:h * hw1 + hd] = bv[h * hd:(h + 1) * hd]
        bva[h * hw1 + hd] = 1.0
    wv_arr = wva.reshape(n_kt, 128, jvc)
    bv_arr = np.broadcast_to(bva[None, :], (128, jvc))

    outw_arr = np.ascontiguousarray(
        out_w[:, rows].T.reshape(n_ct, 128, d))

    inv_freq = 1.0 / (ROPE_BASE ** (np.arange(0, rd, 2, dtype=np.float32) / rd))
    t = np.arange(s, dtype=np.float32)
    freqs = np.outer(t, inv_freq)  # [s, rh]
    # [128, n_st, nh, rh]: value depends on (token=st*128+p, freq i); repl. nh
    cos_arr = np.cos(freqs).astype(np.float32).reshape(n_st, 128, rh)
    cos_arr = np.broadcast_to(cos_arr[:, :, None, :], (n_st, 128, nh, rh))
    cos_arr = np.ascontiguousarray(
        cos_arr.transpose(1, 0, 2, 3).reshape(128, n_st * nh * rh))
    sin_arr = np.sin(freqs).astype(np.float32).reshape(n_st, 128, rh)
    sin_arr = np.broadcast_to(sin_arr[:, :, None, :], (n_st, 128, nh, rh))
    sin_arr = np.ascontiguousarray(
        sin_arr.transpose(1, 0, 2, 3).reshape(128, n_st * nh * rh))

    km = np.arange(128)[:, None]
    qm = np.arange(128)[None, :]
    mask_arr = (qm >= km).astype(np.float32)

    xa = np.ascontiguousarray(x[bi].T)
    xs_arr = np.ascontiguousarray(
        xa.reshape(n_kt, 128, n_st, 128).transpose(2, 1, 0, 3)
    ).reshape(n_st, 128, n_kt * 128)

    return {
        "xs": xs_arr.astype(npdt),
        "wqk": np.ascontiguousarray(wqk_arr).astype(npdt),
        "wv": np.ascontiguousarray(wv_arr).astype(npdt),
        "outw": outw_arr.astype(npdt),
        "cosR": cos_arr.astype(npdt),
        "sinR": sin_arr.astype(npdt),
        "masks": np.ascontiguousarray(mask_arr).astype(npdt),
        "bqk": np.ascontiguousarray(bqk_arr).astype(npdt),
        "bv": np.ascontiguousarray(bv_arr).astype(npdt),
    }


_CACHE = {}


def run_mha(cfg, x, wqkv_w, wqkv_b, out_w, out_b, trace=False):
    key = tuple(sorted(cfg.items()))
    if key not in _CACHE:
        _CACHE[key] = build_program(cfg)
    nc = _CACHE[key]
    in_maps = [
        prep_core_inputs(cfg, x, wqkv_w, wqkv_b, out_w, c)
        for c in range(N_CORES)
    ]
    res = run_bass_kernel_spmd(nc, in_maps, core_ids=list(range(N_CORES)),
                               trace=trace)
    d, s = cfg["d"], cfg["s"]
    y = np.zeros((B, s, d), np.float32)
    for bi in range(B):
        acc = np.zeros((d, s), np.float32)
        for c in range(bi * NBG, (bi + 1) * NBG):
            acc += res.results[c]["y"].astype(np.float32)
        y[bi] = acc.T + out_b[None, :]
    return y, res


def kernel(x, wqkv_w, wqkv_b, out_w, out_b):
    cfg = make_cfg(dt=os.environ.get("KMHA_DT", "bf16"))
    y, _ = run_mha(cfg, np.asarray(x, np.float32),
                   np.asarray(wqkv_w, np.float32),
                   np.asarray(wqkv_b, np.float32),
                   np.asarray(out_w, np.float32),
                   np.asarray(out_b, np.float32))
    return y
